# revision 2
# baseline (speedup 1.0000x reference)
"""Trainium2 Bass kernel for the Camera ISP pipeline (mosaic -> gaussian blur
-> per-channel piecewise-linear calibration -> noise -> Malvar demosaic -> clip).

v4 strategy (per core, pure data parallel over batch: 4 images/core):
- Host ships the 4 bayer quad planes (polyphase components) of each image as
  fp16, partition-major, so input DMA is 4x smaller than v3's f32 planes and
  every descriptor is a contiguous 1KiB run. Noise ships the same way
  (pre-scaled 1/255); the output travels back as fp16 quad planes that the
  host re-interleaves.
- Vertical blur as fp16 banded matmuls on PE directly in the quad domain,
  ACT evacuation to fp16, horizontal blur as Pool pair-sum + DVE combine
  (fp16 throughout; the LUT domain [0,16] tolerates fp16 quantization).
- 17-knot np.interp: chained custom DVE instructions (2 piecewise segments
  each); G quads share one chain. The last OFFLOAD_C kinks run as ACT relu
  planes accumulated by PE diagonal bands into PSUM together with the noise
  and the DVE part (identity bands), so the noisy linear image (nyr) is
  assembled in PSUM and evacuated once by ACT.
- Malvar 5x5 demosaic as fp16 banded matmuls accumulating in PSUM.
- Clip/evac into quad-plane output tiles (contiguous stores -> DVE 4x mode
  eligible); 3 output DMAs per image issued from ACT's queue so SP's
  input-load queue never blocks behind them.
- Software-pipelined emission: the next image's blur front is emitted
  between this image's interp chains and its demosaic, so PE/ACT/Pool keep
  working while DVE runs the serial chains.
"""

import sys

sys.path.insert(0, "/opt/trn_rl_repo")

import numpy as np

import concourse.bacc as bacc
import concourse.bass as bass
import concourse.tile as tile
from concourse import mybir
from concourse.bass_utils import run_bass_kernel_spmd
from concourse import dve_ops as _dops
from concourse.dve_spec import (
    C0, C1, C2, C3, One, Spec, Src0, Src1,
    _has_src1, _spill_c3_to_src1, lower, relu,
)
from concourse.dve_uop import DveOpSpec

F32 = mybir.dt.float32
F32R = mybir.dt.float32r
F16 = mybir.dt.float16
AOT = mybir.AluOpType
ACT_F = mybir.ActivationFunctionType

B_TOT, H, W = 32, 512, 512
N_CORES = 8
B_LOC = B_TOT // N_CORES           # images per core
Q = H // 2                          # quad-plane dim (256)
NPAGE = Q // 128                    # pages per quad plane (2)
DELTA = 255.0 / 16.0                # knot spacing of the LUT

# ---- tuning knobs ---------------------------------------------------------
OFFLOAD_C = 2          # trailing kinks on ACT+PE instead of DVE (even)
# clip mode for the 16 demosaic tiles:
#   "act+pool": ACT relu(ps) + Pool min(t,1)
#   "act+dve": ACT relu(ps) + DVE min(t,1)
#   "dve1"   : single DVE tensor_scalar max0,min1 from PSUM
DEM_CLIP_MODE = ["act+pool"] * 16
# engine for each of the 8 raw clip tiles ("gpsimd" or "vector")
RAW_CLIP_ENGINE = ["gpsimd"] * 8


# ---------------------------------------------------------------------------
# custom DVE ops (2 LUT segments per instruction)
# ---------------------------------------------------------------------------

def _head_ref(in0, in1, s0, s1, imm2):
    p = in0.shape[0]
    x = np.asarray(in0, np.float32).reshape(p, -1)
    d1 = np.asarray(in1, np.float32).reshape(p, 1)
    return (s0 * x + s1) + d1 * np.maximum(x - 1.0, 0.0)


def _pair_ref(in0, in1, s0, s1, imm2):
    x = np.asarray(in0, np.float32)
    acc = np.asarray(in1, np.float32).reshape(x.shape)
    return (acc + s0 * np.maximum(x - imm2, 0.0)) + s1 * np.maximum(
        x - imm2 - 1.0, 0.0
    )


def _register_op(name, spec):
    for op in _dops.OPS:
        if op.name == name:
            return op
    row = _dops._CUSTOM_DVE_ROW_BASE + len(_dops.OPS)
    assert row < 0x20, "custom DVE opcode rows exhausted"
    _dops._SUB_OPCODE_FOR_NAME[name] = row
    shas = {}
    for ver in ("v3", "v4"):
        try:
            s = DveOpSpec(name=name, opcode=row, uops=lower(spec, ver=ver),
                          rd1_en=_has_src1(spec))
            shas[ver] = s.sha(ver)
        except Exception:
            pass
    op = _dops.DveOp(name, spec, subdim=False, uops_sha=shas)
    _dops.OPS.append(op)
    _dops.CUSTOM_DVE_SPECS[name] = spec
    return op


INTERP_HEAD = _register_op(
    "CAM_INTERP_HEAD",
    Spec(body=_spill_c3_to_src1((C0 * Src0 + C1) + C3 * relu(Src0 - One)),
         reference=_head_ref),
)
INTERP_PAIR = _register_op(
    "CAM_INTERP_PAIR",
    Spec(body=(Src1 + C0 * relu(Src0 - C2)) + C1 * relu(Src0 - (C2 + One)),
         reference=_pair_ref),
)


# ---------------------------------------------------------------------------
# host-side constant planning
# ---------------------------------------------------------------------------

def _gauss1d(sigma=0.4):
    x = np.array([-1.0, 0.0, 1.0], dtype=np.float64)
    g = np.exp(-(x ** 2) / (2.0 * sigma ** 2))
    g /= g.sum()
    return g.astype(np.float32)  # [g0, g1, g0]


_G_AT = np.array([[0, 0, -1, 0, 0], [0, 0, 2, 0, 0], [-1, 2, 4, 2, -1],
                  [0, 0, 2, 0, 0], [0, 0, -1, 0, 0]], np.float32) / 8.0
_K_H = np.array([[0, 0, 0.5, 0, 0], [0, -1, 0, -1, 0], [-1, 4, 5, 4, -1],
                 [0, -1, 0, -1, 0], [0, 0, 0.5, 0, 0]], np.float32) / 8.0
_K_V = _K_H.T.copy()
_K_D = np.array([[0, 0, -1.5, 0, 0], [0, 2, 0, 2, 0], [-1.5, 0, 6, 0, -1.5],
                 [0, 2, 0, 2, 0], [0, 0, -1.5, 0, 0]], np.float32) / 8.0
FILTS = {"G": _G_AT, "H": _K_H, "V": _K_V, "D": _K_D}

CONV_OUT = [
    ((0, 0), "V", 0), ((0, 1), "D", 0),
    ((0, 0), "H", 2), ((0, 1), "G", 1),
    ((1, 0), "G", 1), ((1, 1), "H", 0),
    ((1, 0), "D", 2), ((1, 1), "V", 2),
]
RAW_OUT = {(0, 0): 1, (0, 1): 2, (1, 0): 0, (1, 1): 1}  # quad -> raw channel

# quad planes: 0=G1(even,even) 1=B(even,odd) 2=R(odd,even) 3=G2(odd,odd)
QUADS = ((0, 0), (0, 1), (1, 0), (1, 1))
QUAD_CH = {0: 1, 1: 2, 2: 0, 3: 1}   # quad idx -> lut channel (G,B,R,G)
# CMAP channel picked at each quad (bayer 'gbrg')
QUAD_SRC_CH = {0: 1, 1: 2, 2: 0, 3: 1}
# load order: vblur family 0 needs planes 0 (cen) and 2 (nei) first
LOAD_ORDER = [0, 2, 1, 3]


class _BandBuilder:
    def __init__(self):
        self.mats = []
        self._idx = {}

    def add(self, m):
        key = m.tobytes()
        if key not in self._idx:
            self._idx[key] = len(self.mats)
            self.mats.append(m.copy())
        return self._idx[key]


def build_plan(yp):
    """All host-derived constants. yp: [3,17] float32 (255-domain)."""
    yp = np.asarray(yp, np.float32)
    g = _gauss1d()
    g0, g1 = float(g[0]), float(g[1])
    scale_v = 255.0 * g1 / DELTA
    rho = g0 / g1
    cd, co = scale_v * g1, scale_v * g0

    I = np.eye(128, dtype=np.float32)
    sub = np.zeros((128, 128), np.float32)
    for m in range(1, 128):
        sub[m - 1, m] = 1.0
    sup = np.zeros((128, 128), np.float32)
    for m in range(127):
        sup[m + 1, m] = 1.0

    bb = _BandBuilder()

    b_cen = bb.add(cd * I)
    m_up0 = co * (I + sub)
    m_up0[0, 0] += co
    b_up0 = bb.add(m_up0)
    b_up1 = bb.add(co * (I + sub))
    m_upx = np.zeros((128, 128), np.float32)
    m_upx[127, 0] = co
    b_upx = bb.add(m_upx)
    b_dn0 = bb.add(co * (I + sup))
    m_dnx = np.zeros((128, 128), np.float32)
    m_dnx[0, 127] = co
    b_dnx = bb.add(m_dnx)
    m_dn1 = co * (I + sup)
    m_dn1[127, 127] += co
    b_dn1 = bb.add(m_dn1)

    # vblur families: quad qi -> (cen plane, nei plane, direction)
    vb_fams = {
        0: (0, 2, "up"), 1: (1, 3, "up"),
        2: (2, 0, "dn"), 3: (3, 1, "dn"),
    }
    vb_mm = {}
    for qi, (cen, nei, d) in vb_fams.items():
        pages = []
        for pg in range(NPAGE):
            mm = [(b_cen, cen, pg)]
            if d == "up":
                if pg == 0:
                    mm.append((b_up0, nei, 0))
                else:
                    mm.append((b_up1, nei, 1))
                    mm.append((b_upx, nei, 0))
            else:
                if pg == 0:
                    mm.append((b_dn0, nei, 0))
                    mm.append((b_dnx, nei, 1))
                else:
                    mm.append((b_dn1, nei, 1))
            pages.append(mm)
        vb_mm[qi] = pages

    def reflected_src(i_src, pr):
        if 0 <= i_src < Q:
            return i_src
        y_src = 2 * i_src + pr
        y_r = -y_src if y_src < 0 else 2 * (H - 1) - y_src
        assert y_r % 2 == pr
        return y_r // 2

    def emit_groups(groups, page):
        mats = {}
        for (plane, sj), terms in sorted(groups.items()):
            for m in range(128):
                i_out = 128 * page + m
                for si, w, pr in terms:
                    i_src = reflected_src(i_out + si, pr)
                    sp, k = i_src // 128, i_src % 128
                    key = (plane, sp, sj)
                    if key not in mats:
                        mats[key] = np.zeros((128, 128), np.float32)
                    mats[key][k, m] += w
        return [(bb.add(mat), plane, sp, sj)
                for (plane, sp, sj), mat in sorted(mats.items(),
                                                   key=lambda x: x[0])]

    dem = []
    for (r, c), fname, ch in CONV_OUT:
        K = FILTS[fname]
        groups = {}
        for dy in range(-2, 3):
            for dx in range(-2, 3):
                w = float(K[2 + dy, 2 + dx])
                if w == 0.0:
                    continue
                pr = (r + dy) % 2
                si = (r + dy - pr) // 2
                pc = (c + dx) % 2
                sj = (c + dx - pc) // 2
                plane = 2 * pr + pc
                groups.setdefault((plane, sj), []).append((si, w, pr))
        pages = [emit_groups(groups, page) for page in range(NPAGE)]
        dem.append(((r, c), fname, ch, pages))

    yps = yp / 255.0
    interp = []
    for ch in range(3):
        y0 = float(yps[ch, 0])
        s = np.diff(yps[ch]).astype(np.float64)
        d = np.diff(s)
        interp.append({
            "y0": y0, "s0": float(s[0]),
            "d": [float(v) for v in d],
        })

    off_bands = {}
    ident16 = bb.add(I)
    for ch in range(3):
        for k in range(16 - OFFLOAD_C, 16):
            off_bands[(ch, k)] = bb.add(
                np.float32(interp[ch]["d"][k - 1]) * I)

    bands16 = np.stack(bb.mats).astype(np.float16)
    bands32 = np.eye(128, dtype=np.float32)[None]  # identity for acc matmul
    return {
        "bands16": bands16, "bands32": bands32,
        "vb_mm": vb_mm, "dem": dem, "interp": interp, "rho": rho,
        "off_bands": off_bands, "ident16": ident16, "ident32": 0,
    }


# ---------------------------------------------------------------------------
# kernel builder
# ---------------------------------------------------------------------------

def build_kernel(plan):
    nc = bacc.Bacc(None, target_bir_lowering=False, debug=False)
    # quad-plane fp16 inputs, partition-major: [b, plane, p, page, col]
    im = nc.dram_tensor("im", [B_LOC, 4, 128, NPAGE, Q], F16,
                        kind="ExternalInput").ap()
    noise = nc.dram_tensor("noise", [B_LOC, 4, 128, NPAGE, Q], F16,
                           kind="ExternalInput").ap()
    n16 = plan["bands16"].shape[0]
    n32 = plan["bands32"].shape[0]
    bands16_d = nc.dram_tensor("bands16", [n16, 128, 128], F16,
                               kind="ExternalInput").ap()
    bands32_d = nc.dram_tensor("bands32", [n32, 128, 128], F32R,
                               kind="ExternalInput").ap()
    out = nc.dram_tensor("out", [B_LOC, 3, 4, 128, NPAGE, Q], F16,
                         kind="ExternalOutput").ap()

    rho = plan["rho"]
    itp = plan["interp"]
    n_kink_dve = 15 - OFFLOAD_C
    assert (n_kink_dve - 1) % 2 == 0, "OFFLOAD_C must be even"
    n_pairs = (n_kink_dve - 1) // 2

    from contextlib import ExitStack
    with tile.TileContext(nc) as tc, ExitStack() as ctx:
        consts = ctx.enter_context(tc.tile_pool(name="consts", bufs=1))
        imp = ctx.enter_context(tc.tile_pool(name="imp", bufs=2))
        nsp = ctx.enter_context(tc.tile_pool(name="nsp", bufs=2))
        sxp = ctx.enter_context(tc.tile_pool(name="sxp", bufs=2))
        xtp = ctx.enter_context(tc.tile_pool(name="xtp", bufs=2))
        accp = ctx.enter_context(tc.tile_pool(name="accp", bufs=2))
        nyr = ctx.enter_context(tc.tile_pool(name="nyr", bufs=2))
        rlp = ctx.enter_context(tc.tile_pool(name="rlp", bufs=2))
        cvp = ctx.enter_context(tc.tile_pool(name="cvp", bufs=4))
        outp = ctx.enter_context(tc.tile_pool(name="outp", bufs=2))
        psum_vb = ctx.enter_context(
            tc.tile_pool(name="psvb", bufs=1, space="PSUM"))
        psum_ac = ctx.enter_context(
            tc.tile_pool(name="psac", bufs=4, space="PSUM"))
        psum_dm = ctx.enter_context(
            tc.tile_pool(name="psdm", bufs=2, space="PSUM"))

        # --- constants ---
        b16_all = consts.tile([128, n16, 128], F16, tag="bands16")
        nc.sync.dma_start(out=b16_all,
                          in_=bands16_d.rearrange("n k m -> k n m"))
        band16_t = [b16_all[:, i, :] for i in range(n16)]
        b32_all = consts.tile([128, n32, 128], F32R, tag="bands32")
        nc.sync.dma_start(out=b32_all,
                          in_=bands32_d.rearrange("n k m -> k n m"))
        band32_t = [b32_all[:, i, :] for i in range(n32)]
        d1_t = consts.tile([128, 3], F32, tag="d1")
        for ch in range(3):
            nc.vector.memset(d1_t[:, ch:ch + 1], itp[ch]["d"][0])
        kb_t = consts.tile([128, max(OFFLOAD_C, 1)], F32, tag="kbias")
        for i, k in enumerate(range(16 - OFFLOAD_C, 16)):
            nc.vector.memset(kb_t[:, i:i + 1], -float(k))
        one_t = consts.tile([128, 1], F32, tag="one")
        nc.vector.memset(one_t, 1.0)

        def engine_of(name):
            return {"act": nc.scalar, "gpsimd": nc.gpsimd,
                    "vector": nc.vector}[name]

        def emit_front(b):
            qt = [None] * 4
            for pi in LOAD_ORDER:
                t = imp.tile([128, NPAGE, Q], F16, tag=f"plane{pi}",
                             name=f"pl{b}_{pi}")
                nc.sync.dma_start(
                    out=t, in_=im[b, pi])
                qt[pi] = t
            noi = nsp.tile([128, 4, NPAGE, Q], F16, tag="noise",
                           name=f"ns{b}")
            nc.sync.dma_start(out=noi,
                              in_=noise[b].rearrange("q p t w -> p q t w"))
            return qt, noi

        def emit_blur(b, front):
            """vblur (PE) -> evac (ACT) -> hblur (Pool pair + DVE stt)."""
            qt, _noi = front
            vbt = {}
            for qi in range(4):
                ps = psum_vb.tile([128, NPAGE, Q], F32,
                                  tag=f"vb{'AB'[qi % 2]}",
                                  name=f"vps{b}_{qi}")
                for pg, mm in enumerate(plan["vb_mm"][qi]):
                    for i, (bidx, plane, spage) in enumerate(mm):
                        nc.tensor.matmul(
                            ps[:, pg, :], band16_t[bidx],
                            qt[plane][:, spage, :],
                            start=(i == 0), stop=(i == len(mm) - 1))
                vs = sxp.tile([128, NPAGE, Q], F16, tag=f"vbs{qi}",
                              name=f"vbs{b}_{qi}")
                nc.scalar.copy(out=vs[:], in_=ps[:])
                vbt[qi] = vs

            x_G = xtp.tile([128, 2 * NPAGE, Q], F16, tag="xG", name=f"xG{b}")
            x_B = xtp.tile([128, NPAGE, Q], F16, tag="xB", name=f"xB{b}")
            x_R = xtp.tile([128, NPAGE, Q], F16, tag="xR", name=f"xR{b}")
            x_of = {0: x_G[:, 0:NPAGE, :], 3: x_G[:, NPAGE:2 * NPAGE, :],
                    1: x_B[:], 2: x_R[:]}
            for qi, (r, c) in enumerate(QUADS):
                cen = vbt[qi]
                nei = vbt[2 * r + (1 - c)]
                s = sxp.tile([128, NPAGE, Q], F16, tag="s", name=f"s{b}_{qi}")
                if c == 0:
                    nc.gpsimd.tensor_add(out=s[:, :, 1:Q],
                                         in0=nei[:, :, 0:Q - 1],
                                         in1=nei[:, :, 1:Q])
                    nc.gpsimd.tensor_scalar_mul(
                        out=s[:, :, 0:1], in0=nei[:, :, 0:1], scalar1=2.0)
                else:
                    nc.gpsimd.tensor_add(out=s[:, :, 0:Q - 1],
                                         in0=nei[:, :, 0:Q - 1],
                                         in1=nei[:, :, 1:Q])
                    nc.gpsimd.tensor_scalar_mul(
                        out=s[:, :, Q - 1:Q], in0=nei[:, :, Q - 1:Q],
                        scalar1=2.0)
                nc.vector.scalar_tensor_tensor(
                    out=x_of[qi], in0=s, scalar=rho, in1=cen[:],
                    op0=AOT.mult, op1=AOT.add)
            return {"x_G": x_G, "x_B": x_B, "x_R": x_R, "x_of": x_of}

        def emit_chains(b, blur):
            """Custom-DVE interp chains + ACT relu planes for offload."""
            def interp_chain(ch, xt, shape_free, tagc):
                co = itp[ch]
                xf = xt.rearrange("p a b -> p (a b)")
                a0 = accp.tile([128, shape_free, Q], F32R, tag=f"a0{tagc}",
                               name=f"a0{b}{tagc}")
                a1 = accp.tile([128, shape_free, Q], F32R, tag=f"a1{tagc}",
                               name=f"a1{b}{tagc}")
                nc.vector._custom_dve(
                    INTERP_HEAD, out=a0[:].rearrange("p a b -> p (a b)"),
                    in0=xf, in1=d1_t[:, ch:ch + 1],
                    s0=co["s0"], s1=co["y0"])
                src, dst = a0, a1
                for j in range(1, n_pairs + 1):
                    nc.vector._custom_dve(
                        INTERP_PAIR,
                        out=dst[:].rearrange("p a b -> p (a b)"),
                        in0=xf,
                        in1=src[:].rearrange("p a b -> p (a b)"),
                        s0=co["d"][2 * j - 1], s1=co["d"][2 * j],
                        imm2=float(2 * j))
                    src, dst = dst, src
                return src

            acc_G = interp_chain(1, blur["x_G"][:], 2 * NPAGE, "G")
            acc_B = interp_chain(2, blur["x_B"][:], NPAGE, "B")
            acc_R = interp_chain(0, blur["x_R"][:], NPAGE, "R")
            acc_of = {0: acc_G[:, 0:NPAGE, :],
                      3: acc_G[:, NPAGE:2 * NPAGE, :],
                      1: acc_B[:], 2: acc_R[:]}

            relus = {}
            for k in range(16 - OFFLOAD_C, 16):
                r_G = rlp.tile([128, 2 * NPAGE, Q], F16, tag="rG",
                               name=f"rG{b}_{k}")
                r_B = rlp.tile([128, NPAGE, Q], F16, tag="rB",
                               name=f"rB{b}_{k}")
                r_R = rlp.tile([128, NPAGE, Q], F16, tag="rR",
                               name=f"rR{b}_{k}")
                kb = kb_t[:, k - (16 - OFFLOAD_C):k - (16 - OFFLOAD_C) + 1]
                nc.scalar.activation(out=r_G[:], in_=blur["x_G"][:],
                                     func=ACT_F.Relu, bias=kb)
                nc.scalar.activation(out=r_B[:], in_=blur["x_B"][:],
                                     func=ACT_F.Relu, bias=kb)
                nc.scalar.activation(out=r_R[:], in_=blur["x_R"][:],
                                     func=ACT_F.Relu, bias=kb)
                relus[k] = {0: r_G[:, 0:NPAGE, :],
                            3: r_G[:, NPAGE:2 * NPAGE, :],
                            1: r_B[:], 2: r_R[:]}
            return acc_of, relus

        def emit_nyr(b, front, acc_of, relus):
            """nyr assembled in PSUM by PE (noise + offload + interp),
            evacuated to fp16 SBUF by ACT, halo cols padded by Pool."""
            _qt, noi = front
            nyrtiles = []
            for qi, (r, c) in enumerate(QUADS):
                pa = psum_ac.tile([128, NPAGE, Q], F32, tag="pacc",
                                  name=f"pa{b}_{qi}")
                for pg in range(NPAGE):
                    nc.tensor.matmul(
                        pa[:, pg, :], band16_t[plan["ident16"]],
                        noi[:, qi, pg, :], start=True, stop=False)
                    for k in sorted(relus):
                        bidx = plan["off_bands"][(QUAD_CH[qi], k)]
                        nc.tensor.matmul(
                            pa[:, pg, :], band16_t[bidx],
                            relus[k][qi][:, pg, :], start=False, stop=False)
                    nc.tensor.matmul(
                        pa[:, pg, :], band32_t[plan["ident32"]],
                        acc_of[qi][:, pg, :], start=False, stop=True)
                npr = nyr.tile([128, NPAGE, Q + 2], F16, tag=f"nyr{qi}",
                               name=f"npr{b}_{qi}")
                nc.scalar.copy(out=npr[:, :, 1:Q + 1], in_=pa[:])
                lsrc = 2 if c == 0 else 1
                rsrc = Q if c == 0 else Q - 1
                nc.gpsimd.tensor_copy(out=npr[:, :, 0:1],
                                      in_=npr[:, :, lsrc:lsrc + 1])
                nc.gpsimd.tensor_copy(out=npr[:, :, Q + 1:Q + 2],
                                      in_=npr[:, :, rsrc:rsrc + 1])
                nyrtiles.append(npr)
            return nyrtiles

        def emit_back(b, nyrtiles):
            """raw clips, demosaic + clip/evac, stores (quad-plane layout)."""
            ot = [outp.tile([128, 4, NPAGE, Q], F16, tag=f"o{ch}",
                            name=f"ot{b}_{ch}") for ch in range(3)]

            for qi, (r, c) in enumerate(QUADS):
                rch = RAW_OUT[(r, c)]
                for pg in range(NPAGE):
                    eng = engine_of(RAW_CLIP_ENGINE[qi * NPAGE + pg])
                    eng.tensor_scalar(
                        out=ot[rch][:, qi, pg, :],
                        in0=nyrtiles[qi][:, pg, 1:Q + 1],
                        scalar1=0.0, scalar2=1.0,
                        op0=AOT.max, op1=AOT.min)

            ci = 0
            for fi, ((r, c), fname, ch, pages) in enumerate(plan["dem"]):
                qi = 2 * r + c
                for page, mains in enumerate(pages):
                    ps = psum_dm.tile([128, Q], F32, tag="dmps",
                                      name=f"dm{b}_{ci}")
                    for i, (bidx, plane, spage, sj) in enumerate(mains):
                        nc.tensor.matmul(
                            ps[:], band16_t[bidx],
                            nyrtiles[plane][:, spage, 1 + sj:1 + sj + Q],
                            start=(i == 0), stop=(i == len(mains) - 1))
                    dst = ot[ch][:, qi, page, :]
                    mode = DEM_CLIP_MODE[ci]
                    if mode == "dve1":
                        nc.vector.tensor_scalar(
                            out=dst, in0=ps[:], scalar1=0.0, scalar2=1.0,
                            op0=AOT.max, op1=AOT.min)
                    else:
                        tcl = cvp.tile([128, Q], F32, tag="conv",
                                       name=f"cv{b}_{ci}")
                        nc.scalar.activation(out=tcl[:], in_=ps[:],
                                             func=ACT_F.Relu)
                        eng = (nc.gpsimd if mode == "act+pool"
                               else nc.vector)
                        eng.tensor_scalar(
                            out=dst, in0=tcl[:], scalar1=1.0,
                            scalar2=None, op0=AOT.min)
                    ci += 1

            # stores from ACT's queue so SP's input loads don't block
            for ch in range(3):
                nc.scalar.dma_start(
                    out=out[b, ch].rearrange("q p t w -> p q t w"),
                    in_=ot[ch][:])

        # ---- software-pipelined emission ----
        fronts = [emit_front(0)]
        fronts.append(emit_front(1))
        blur = emit_blur(0, fronts[0])
        for b in range(B_LOC):
            acc_of, relus = emit_chains(b, blur)
            if b + 1 < B_LOC:
                blur = emit_blur(b + 1, fronts[b + 1])
            if b + 2 < B_LOC:
                fronts.append(emit_front(b + 2))
            nyrtiles = emit_nyr(b, fronts[b], acc_of, relus)
            emit_back(b, nyrtiles)

    nc.compile()
    return nc


# ---------------------------------------------------------------------------
# public entry
# ---------------------------------------------------------------------------

_CACHE = {}


def _get_compiled(yp):
    key = np.asarray(yp, np.float32).tobytes()
    if key not in _CACHE:
        plan = build_plan(yp)
        _CACHE[key] = (build_kernel(plan), plan)
    return _CACHE[key]


def _extract_quads(arr, ch_map):
    """arr: [B, C, H, W] f32 -> [B, 4, 128, NPAGE, Q] fp16 quad planes,
    partition-major. ch_map[qi] = source channel (or 0 for single-channel)."""
    B = arr.shape[0]
    out = np.empty((B, 4, 128, NPAGE, Q), np.float16)
    for qi, (r, c) in enumerate(QUADS):
        pl = arr[:, ch_map[qi], r::2, c::2]          # [B, 256, 256]
        out[:, qi] = pl.reshape(B, NPAGE, 128, Q).transpose(0, 2, 1, 3)
    return out


def build_in_maps(im, yp, noise):
    im = np.asarray(im, np.float32)
    noise = np.asarray(noise, np.float32)
    nc, plan = _get_compiled(np.asarray(yp, np.float32))
    im_q = _extract_quads(im, QUAD_SRC_CH)
    noise_q = _extract_quads(noise * np.float32(1.0 / 255.0),
                             {0: 0, 1: 0, 2: 0, 3: 0})
    in_maps = []
    for k in range(N_CORES):
        sl = slice(k * B_LOC, (k + 1) * B_LOC)
        in_maps.append({
            "im": np.ascontiguousarray(im_q[sl]),
            "noise": np.ascontiguousarray(noise_q[sl]),
            "bands16": plan["bands16"],
            "bands32": plan["bands32"],
        })
    return nc, in_maps


def kernel(im, yp, noise):
    nc, in_maps = build_in_maps(im, yp, noise)
    res = run_bass_kernel_spmd(nc, in_maps, core_ids=list(range(N_CORES)))
    # out: [B_LOC, 3, 4, 128, NPAGE, Q] fp16 quad planes -> [B, 3, H, W]
    oq = np.concatenate(
        [np.asarray(r["out"], np.float16) for r in res.results], axis=0)
    full = np.empty((B_TOT, 3, H, W), np.float32)
    for qi, (r, c) in enumerate(QUADS):
        pl = oq[:, :, qi].astype(np.float32)         # [B, 3, 128, NPAGE, Q]
        full[:, :, r::2, c::2] = pl.transpose(0, 1, 3, 2, 4).reshape(
            B_TOT, 3, Q, Q)
    return full
